# revision 18
# baseline (speedup 1.0000x reference)
"""BertAttention (cross-attention variant) Trainium2 Bass kernel.

Strategy: data-parallel over batch (16 batches -> 8 cores x 2 batches).
Per core, per batch:
  Q^T = Wq^T X^T, K^T = Wk^T C^T (transposed layouts, head-sliced),
  V (natural layout, with an appended ones-column per head for the
  softmax denominator), S^T = K Q^T per head (row-packed pairs of
  heads on the PE), P = exp(S/8) (no max-subtraction needed: scores
  are O(1) by construction), O[q, 65] = P^T(as lhsT) @ V_aug; the
  last column gives the softmax denominator; normalize with a
  reciprocal + free-broadcast multiply on the vector engine.

All matmul operands are bf16 (fp32 PSUM accumulation). All DRAM loads
are contiguous fp32 (HWDGE, big packets); casts run on GpSimd; the
X^T/C^T transposes run on the PE via identity matmuls.
"""

import os
import sys

import numpy as np

for _p in ("/opt/trn_rl_repo", "/root/.axon_site/_ro/trn_rl_repo"):
    if os.path.isdir(_p) and _p not in sys.path:
        sys.path.insert(0, _p)

import concourse.bass as bass  # noqa: E402
import concourse.tile as tile  # noqa: E402
from concourse import bacc, mybir  # noqa: E402
from concourse.bass_utils import run_bass_kernel_spmd  # noqa: E402
from concourse.masks import make_identity  # noqa: E402

# Problem constants (hardcoded per spec)
B, S, D, H, HD = 16, 512, 768, 12, 64
NCORES = 8
BL = B // NCORES  # batches per core = 2
DT = D // 128     # 6 d-tiles
KT = S // 128     # 4 k-token tiles
QT = S // 128     # 4 q-token tiles
HP = H // 2       # 6 head pairs

f32 = mybir.dt.float32
bf16 = mybir.dt.bfloat16
AF = mybir.ActivationFunctionType

_CACHE = {}


def _emit(tc, hs, ct, w_aps, b_aps, out):
    nc = tc.nc
    from contextlib import ExitStack

    with ExitStack() as ctx:
        wpool = ctx.enter_context(tc.tile_pool(name="wpool", bufs=1))

        # ---- identities for PE-transposes ----
        ident_bf = wpool.tile([128, 128], bf16, name="ident_bf")
        make_identity(nc, ident_bf)
        ident_f = wpool.tile([128, 128], f32, name="ident_f")
        make_identity(nc, ident_f)

        psum_p = ctx.enter_context(tc.tile_pool(name="psum_p", bufs=3, space="PSUM"))
        pv_p = ctx.enter_context(tc.tile_pool(name="pv_p", bufs=2, space="PSUM"))

        natp = ctx.enter_context(tc.tile_pool(name="natp", bufs=1))
        xtp = ctx.enter_context(tc.tile_pool(name="xtp", bufs=2))
        qkp = ctx.enter_context(tc.tile_pool(name="qkp", bufs=2))
        vap = ctx.enter_context(tc.tile_pool(name="vap", bufs=2))
        exps_p = ctx.enter_context(tc.tile_pool(name="exps_p", bufs=10))
        orow_p = ctx.enter_context(tc.tile_pool(name="orow_p", bufs=2))
        small_p = ctx.enter_context(tc.tile_pool(name="small_p", bufs=16))


        # ---- batch-0 input loads first: they gate the earliest PE work ----
        early_loads = {}

        def emit_early_loads():
            x_nat = natp.tile([128, QT, D], f32, name="x_nat")
            c_nat = natp.tile([128, QT, D], f32, name="c_nat")
            nc.sync.dma_start(out=c_nat, in_=ct[0].rearrange("(q p) d -> p q d", p=128))
            nc.sync.dma_start(out=x_nat, in_=hs[0].rearrange("(q p) d -> p q d", p=128))
            early_loads[0] = (x_nat, c_nat)

        emit_early_loads()

        # ---- weights: contiguous fp32 HWDGE load + GpSimd cast to bf16 ----
        w_sb = {}
        with tc.tile_pool(name="wstage", bufs=2) as wstage:
            for name in ("v", "q", "k"):
                wt = wpool.tile([128, DT, D], bf16, name=f"w_{name}")
                wr = w_aps[name].rearrange("(a p) d -> p a d", p=128)
                for half in range(2):
                    hd2 = DT // 2
                    wst = wstage.tile([128, hd2, D], f32, name="wst", tag="wst")
                    nc.sync.dma_start(
                        out=wst, in_=wr[:, half * hd2:(half + 1) * hd2, :]
                    )
                    nc.vector.tensor_copy(
                        out=wt[:, half * hd2:(half + 1) * hd2, :], in_=wst
                    )
                w_sb[name] = wt

            # biases for Q^T/K^T: natural [6,128] load, PE-transpose to [128,6]
            bias_sb = {}
            for name in ("q", "k"):
                bn = wstage.tile([DT, 128], f32, name="bn", tag="bn")
                nc.sync.dma_start(
                    out=bn, in_=b_aps[name].rearrange("(a p) -> a p", p=128)
                )
                tpb = psum_p.tile([128, 1024], f32, tag="big", name="tpb")
                nc.tensor.transpose(tpb[:, 0:DT], bn, ident_f[0:DT, 0:DT])
                bsb = wpool.tile([128, DT], f32, name=f"b_{name}")
                nc.vector.tensor_copy(out=bsb, in_=tpb[:, 0:DT])
                bias_sb[name] = bsb

        bv_sb = wpool.tile([128, H, HD], f32, name="bv_sb")
        bv = b_aps["v"]
        bv_bcast = bass.AP(tensor=bv.tensor, offset=bv.offset, ap=[[0, 128], [1, D]])
        nc.gpsimd.dma_start(out=bv_sb, in_=bv_bcast)

        # ---- per-batch input staging: fp32 PE-transpose, cast on the
        #      PSUM->SBUF copy ----
        def stage_loads(b):
            if b in early_loads:
                x_nat, c_nat = early_loads[b]
            else:
                x_nat = natp.tile([128, QT, D], f32, name="x_nat")
                c_nat = natp.tile([128, QT, D], f32, name="c_nat")
                nc.sync.dma_start(
                    out=c_nat, in_=ct[b].rearrange("(q p) d -> p q d", p=128)
                )
                nc.sync.dma_start(
                    out=x_nat, in_=hs[b].rearrange("(q p) d -> p q d", p=128)
                )
            xt = xtp.tile([128, DT, S], bf16, name="xt")
            ctt = xtp.tile([128, DT, S], bf16, name="ctt")
            return x_nat, c_nat, xt, ctt

        def stage_chunks(x_nat, c_nat, xt, ctt):
            chunks = []
            for src, dst in ((c_nat, ctt), (x_nat, xt)):
                for dt_ in range(DT):
                    def f(src=src, dst=dst, dt_=dt_):
                        tp = psum_p.tile([128, 512], f32, tag="big", name="tps")
                        for q in range(QT):
                            nc.tensor.transpose(
                                tp[:, q * 128:(q + 1) * 128],
                                src[:, q, dt_ * 128:(dt_ + 1) * 128],
                                ident_f,
                            )
                        nc.vector.tensor_copy(out=dst[:, dt_, :], in_=tp)
                    chunks.append(f)
            return chunks

        # ---- projection chunk closures for one batch ----
        def proj_alloc(store):
            qt_t = qkp.tile([128, DT, S], bf16, name="qt_t")
            kt_t = qkp.tile([128, DT, S], bf16, name="kt_t")
            va_t = vap.tile([128, KT, H, HD + 1], bf16, name="va_t")
            store["qt"], store["kt"], store["va"] = qt_t, kt_t, va_t

        def qk_chunks(xt, ctt, store):
            """Q^T/K^T chunk closures, ordered (Q0,K0),(Q1,K1),... so pair
            hp only needs the first 2(hp+1) chunks."""
            chunks = []
            for m in range(DT):
                for wname, src, dstT in (("q", xt, store["qt"]), ("k", ctt, store["kt"])):
                    def f(wname=wname, src=src, dstT=dstT, m=m):
                        ps = psum_p.tile([128, 1024], f32, tag="big", name="ps_big")
                        for k in range(DT):
                            nc.tensor.matmul(
                                ps[:, 0:S],
                                lhsT=w_sb[wname][:, k, m * 128:(m + 1) * 128],
                                rhs=src[:, k, :],
                                start=(k == 0),
                                stop=(k == DT - 1),
                            )
                        nc.vector.tensor_scalar_add(
                            out=dstT[:, m, :],
                            in0=ps[:, 0:S],
                            scalar1=bias_sb[wname][:, m:m + 1],
                        )
                    chunks.append(f)
            return chunks

        def v_chunks(ctt, store):
            chunks = []
            va_t = store["va"]
            for m in range(KT):
                def f(m=m):
                    ps = psum_p.tile([128, 1024], f32, tag="big", name="ps_big")
                    for lo, hi in ((0, 512), (512, 768)):
                        for k in range(DT):
                            nc.tensor.matmul(
                                ps[:, lo:hi],
                                lhsT=ctt[:, k, m * 128:(m + 1) * 128],
                                rhs=w_sb["v"][:, k, lo:hi],
                                start=(k == 0),
                                stop=(k == DT - 1),
                            )
                    ps_h = ps[:, 0:D].rearrange("p (h x) -> p h x", x=HD)
                    nc.vector.tensor_add(out=va_t[:, m, :, 0:HD], in0=ps_h, in1=bv_sb)
                    nc.vector.memset(va_t[:, m, :, HD:HD + 1], 1.0)
                chunks.append(f)
            return chunks

        # ---- one attention head-pair for one batch ----
        def attn_pair(store, hp, orow):
            qt_t, kt_t, va_t = store["qt"], store["kt"], store["va"]
            exps_tiles = []
            for kt in range(KT):
                st = psum_p.tile([128, 2, S], f32, tag="big", name="st")
                for pr in (0, 1):
                    nc.tensor.matmul(
                        st[:, pr, :],
                        lhsT=kt_t[pr * 64:(pr + 1) * 64, hp, kt * 128:(kt + 1) * 128],
                        rhs=qt_t[pr * 64:(pr + 1) * 64, hp, :],
                        start=True,
                        stop=True,
                        tile_position=(pr * 64, 0),
                    )
                ex = exps_p.tile([128, 2, S], bf16, name="ex")
                nc.scalar.activation(out=ex, in_=st, func=AF.Exp, scale=0.125)
                exps_tiles.append(ex)
            for pr in (0, 1):
                h = 2 * hp + pr
                pv = pv_p.tile([128, QT, HD + 1], f32, tag="pv", name="pv")
                for q in range(QT):
                    for kt in range(KT):
                        nc.tensor.matmul(
                            pv[:, q, :],
                            lhsT=exps_tiles[kt][:, pr, q * 128:(q + 1) * 128],
                            rhs=va_t[:, kt, h, :],
                            start=(kt == 0),
                            stop=(kt == KT - 1),
                        )
                rc = small_p.tile([128, QT], f32, name="rc")
                nc.vector.reciprocal(
                    rc, pv[:, :, HD:HD + 1].rearrange("p a b -> p (a b)")
                )
                rc_b = bass.AP(
                    tensor=rc.tensor,
                    offset=rc.offset,
                    ap=[list(rc.ap[0]), [1, QT], [0, HD]],
                )
                nc.vector.tensor_mul(
                    out=orow[:, :, h * HD:(h + 1) * HD],
                    in0=pv[:, :, 0:HD],
                    in1=rc_b,
                )

        # ---- schedule: start attention as early as possible (ACT needs a
        #      long window); feed Q/K projection chunks just-in-time and
        #      spread batch-1 staging across batch-0's attention pairs ----
        stores = [{}, {}]
        x_nat0, c_nat0, xt0, ct0 = stage_loads(0)
        x_nat1, c_nat1, xt1, ct1 = stage_loads(1)
        for f in stage_chunks(x_nat0, c_nat0, xt0, ct0):
            f()
        proj_alloc(stores[0])
        proj_alloc(stores[1])
        for f in v_chunks(ct0, stores[0]):
            f()
        qk0 = qk_chunks(xt0, ct0, stores[0])
        qk0.pop(0)()
        qk0.pop(0)()

        fillers = stage_chunks(x_nat1, c_nat1, xt1, ct1) + v_chunks(
            ct1, stores[1]
        )

        orow0 = orow_p.tile([128, QT, D], f32, name="orow")
        for hp in range(HP):
            attn_pair(stores[0], hp, orow0)
            if qk0:
                qk0.pop(0)()
                qk0.pop(0)()
            for _ in range(3):
                if fillers:
                    fillers.pop(0)()
        while fillers:
            fillers.pop(0)()
        nc.sync.dma_start(
            out=out[0].rearrange("(q p) d -> p q d", p=128), in_=orow0
        )

        qk1 = qk_chunks(xt1, ct1, stores[1])
        qk1.pop(0)()
        qk1.pop(0)()
        orow1 = orow_p.tile([128, QT, D], f32, name="orow")
        for hp in range(HP):
            attn_pair(stores[1], hp, orow1)
            if qk1:
                qk1.pop(0)()
                qk1.pop(0)()
        nc.sync.dma_start(
            out=out[1].rearrange("(q p) d -> p q d", p=128), in_=orow1
        )


def build_program():
    if "nc" in _CACHE:
        return _CACHE["nc"]
    nc = bacc.Bacc("TRN2", target_bir_lowering=False, debug=False)
    hs = nc.dram_tensor("hs", [BL, S, D], f32, kind="ExternalInput").ap()
    ct = nc.dram_tensor("ct", [BL, S, D], f32, kind="ExternalInput").ap()
    w_aps = {
        n: nc.dram_tensor(f"w{n}", [D, D], f32, kind="ExternalInput").ap()
        for n in ("q", "k", "v")
    }
    b_aps = {
        n: nc.dram_tensor(f"b{n}", [D], f32, kind="ExternalInput").ap()
        for n in ("q", "k", "v")
    }
    out = nc.dram_tensor("out", [BL, S, D], f32, kind="ExternalOutput").ap()
    with tile.TileContext(nc) as tc:
        _emit(tc, hs, ct, w_aps, b_aps, out)
    nc.compile()
    _CACHE["nc"] = nc
    return nc


def make_in_maps(hidden_states, context, Wq, bq, Wk, bk, Wv, bv):
    hidden_states = np.ascontiguousarray(np.asarray(hidden_states, np.float32))
    context = np.ascontiguousarray(np.asarray(context, np.float32))
    common = {
        "wq": np.ascontiguousarray(np.asarray(Wq, np.float32)),
        "wk": np.ascontiguousarray(np.asarray(Wk, np.float32)),
        "wv": np.ascontiguousarray(np.asarray(Wv, np.float32)),
        "bq": np.ascontiguousarray(np.asarray(bq, np.float32)),
        "bk": np.ascontiguousarray(np.asarray(bk, np.float32)),
        "bv": np.ascontiguousarray(np.asarray(bv, np.float32)),
    }
    in_maps = []
    for c in range(NCORES):
        m = dict(common)
        m["hs"] = np.ascontiguousarray(hidden_states[c * BL:(c + 1) * BL])
        m["ct"] = np.ascontiguousarray(context[c * BL:(c + 1) * BL])
        in_maps.append(m)
    return in_maps


def run(in_maps, **kwargs):
    nc = build_program()
    return run_bass_kernel_spmd(nc, in_maps, core_ids=list(range(NCORES)), **kwargs)


def kernel(hidden_states, context, Wq, bq, Wk, bk, Wv, bv):
    in_maps = make_in_maps(hidden_states, context, Wq, bq, Wk, bk, Wv, bv)
    res = run(in_maps)
    outs = [np.asarray(res.results[i]["out"], np.float32) for i in range(NCORES)]
    return np.concatenate(outs, axis=0)


# revision 19
# speedup vs baseline: 1.0854x; 1.0854x over previous
"""BertAttention (cross-attention variant) Trainium2 Bass kernel.

Strategy: data-parallel over batch (16 batches -> 8 cores x 2 batches).
Per core, per batch:
  Q^T = Wq^T X^T, K^T = Wk^T C^T (transposed layouts, head-sliced),
  V (natural layout, with an appended ones-column per head for the
  softmax denominator), S^T = K Q^T per head (row-packed pairs of
  heads on the PE), P = exp(S/8) (no max-subtraction needed: scores
  are O(1) by construction), O[q, 65] = P^T(as lhsT) @ V_aug; the
  last column gives the softmax denominator; normalize with a
  reciprocal + free-broadcast multiply on the vector engine.

All matmul operands are bf16 (fp32 PSUM accumulation). All DRAM loads
are contiguous fp32 (HWDGE, big packets); casts run on GpSimd; the
X^T/C^T transposes run on the PE via identity matmuls.
"""

import os
import sys

import numpy as np

for _p in ("/opt/trn_rl_repo", "/root/.axon_site/_ro/trn_rl_repo"):
    if os.path.isdir(_p) and _p not in sys.path:
        sys.path.insert(0, _p)

import concourse.bass as bass  # noqa: E402
import concourse.tile as tile  # noqa: E402
from concourse import bacc, mybir  # noqa: E402
from concourse.bass_utils import run_bass_kernel_spmd  # noqa: E402
from concourse.masks import make_identity  # noqa: E402

# Problem constants (hardcoded per spec)
B, S, D, H, HD = 16, 512, 768, 12, 64
NCORES = 8
BL = B // NCORES  # batches per core = 2
DT = D // 128     # 6 d-tiles
KT = S // 128     # 4 k-token tiles
QT = S // 128     # 4 q-token tiles
HP = H // 2       # 6 head pairs

f32 = mybir.dt.float32
bf16 = mybir.dt.bfloat16
AF = mybir.ActivationFunctionType

_CACHE = {}


def _emit(tc, hs, ct, w_aps, b_aps, out):
    nc = tc.nc
    from contextlib import ExitStack

    with ExitStack() as ctx:
        wpool = ctx.enter_context(tc.tile_pool(name="wpool", bufs=1))

        # ---- identities for PE-transposes ----
        ident_bf = wpool.tile([128, 128], bf16, name="ident_bf")
        make_identity(nc, ident_bf)
        ident_f = wpool.tile([128, 128], f32, name="ident_f")
        make_identity(nc, ident_f)

        psum_p = ctx.enter_context(tc.tile_pool(name="psum_p", bufs=3, space="PSUM"))
        pv_p = ctx.enter_context(tc.tile_pool(name="pv_p", bufs=2, space="PSUM"))

        natp = ctx.enter_context(tc.tile_pool(name="natp", bufs=1))
        xtp = ctx.enter_context(tc.tile_pool(name="xtp", bufs=2))
        qkp = ctx.enter_context(tc.tile_pool(name="qkp", bufs=2))
        vap = ctx.enter_context(tc.tile_pool(name="vap", bufs=2))
        exps_p = ctx.enter_context(tc.tile_pool(name="exps_p", bufs=10))
        orow_p = ctx.enter_context(tc.tile_pool(name="orow_p", bufs=2))
        small_p = ctx.enter_context(tc.tile_pool(name="small_p", bufs=16))


        # ---- batch-0 input loads first: they gate the earliest PE work ----
        early_loads = {}

        def emit_early_loads():
            x_nat = natp.tile([128, QT, D], f32, name="x_nat")
            c_nat = natp.tile([128, QT, D], f32, name="c_nat")
            nc.sync.dma_start(out=c_nat, in_=ct[0].rearrange("(q p) d -> p q d", p=128))
            nc.sync.dma_start(out=x_nat, in_=hs[0].rearrange("(q p) d -> p q d", p=128))
            early_loads[0] = (x_nat, c_nat)

        emit_early_loads()

        # ---- weights: contiguous fp32 HWDGE load + DVE cast to bf16.
        #      Bias loads are tiny: issue them before the bulk W transfers.
        w_sb = {}
        bias_sb = {}
        bias_nat = {}
        wstage = ctx.enter_context(tc.tile_pool(name="wstage", bufs=2))
        for name in ("q", "k"):
            bn = wstage.tile([DT, 128], f32, name="bn", tag="bn")
            nc.sync.dma_start(
                out=bn, in_=b_aps[name].rearrange("(a p) -> a p", p=128)
            )
            bias_nat[name] = bn
        for name in ("v", "q", "k"):
            wt = wpool.tile([128, DT, D], bf16, name=f"w_{name}")
            wr = w_aps[name].rearrange("(a p) d -> p a d", p=128)
            for half in range(2):
                hd2 = DT // 2
                wst = wstage.tile([128, hd2, D], f32, name="wst", tag="wst")
                nc.sync.dma_start(
                    out=wst, in_=wr[:, half * hd2:(half + 1) * hd2, :]
                )
                nc.vector.tensor_copy(
                    out=wt[:, half * hd2:(half + 1) * hd2, :], in_=wst
                )
            w_sb[name] = wt

        def emit_bias_transposes():
            # PE-transpose [6,128] -> [128,6]; emitted after the staging
            # transposes so the PE queue head never blocks on bias DMAs.
            for name in ("q", "k"):
                tpb = psum_p.tile([128, 1024], f32, tag="big", name="tpb")
                nc.tensor.transpose(
                    tpb[:, 0:DT], bias_nat[name], ident_f[0:DT, 0:DT]
                )
                bsb = wpool.tile([128, DT], f32, name=f"b_{name}")
                nc.vector.tensor_copy(out=bsb, in_=tpb[:, 0:DT])
                bias_sb[name] = bsb

        bv_sb = wpool.tile([128, H, HD], f32, name="bv_sb")
        bv = b_aps["v"]
        bv_bcast = bass.AP(tensor=bv.tensor, offset=bv.offset, ap=[[0, 128], [1, D]])
        nc.gpsimd.dma_start(out=bv_sb, in_=bv_bcast)

        # ---- per-batch input staging: fp32 PE-transpose, cast on the
        #      PSUM->SBUF copy ----
        def stage_loads(b):
            if b in early_loads:
                x_nat, c_nat = early_loads[b]
            else:
                x_nat = natp.tile([128, QT, D], f32, name="x_nat")
                c_nat = natp.tile([128, QT, D], f32, name="c_nat")
                nc.sync.dma_start(
                    out=c_nat, in_=ct[b].rearrange("(q p) d -> p q d", p=128)
                )
                nc.sync.dma_start(
                    out=x_nat, in_=hs[b].rearrange("(q p) d -> p q d", p=128)
                )
            xt = xtp.tile([128, DT, S], bf16, name="xt")
            ctt = xtp.tile([128, DT, S], bf16, name="ctt")
            return x_nat, c_nat, xt, ctt

        def stage_chunks(x_nat, c_nat, xt, ctt):
            chunks = []
            for src, dst in ((c_nat, ctt), (x_nat, xt)):
                for dt_ in range(DT):
                    def f(src=src, dst=dst, dt_=dt_):
                        tp = psum_p.tile([128, 512], f32, tag="big", name="tps")
                        for q in range(QT):
                            nc.tensor.transpose(
                                tp[:, q * 128:(q + 1) * 128],
                                src[:, q, dt_ * 128:(dt_ + 1) * 128],
                                ident_f,
                            )
                        nc.vector.tensor_copy(out=dst[:, dt_, :], in_=tp)
                    chunks.append(f)
            return chunks

        # ---- projection chunk closures for one batch ----
        def proj_alloc(store):
            qt_t = qkp.tile([128, DT, S], bf16, name="qt_t")
            kt_t = qkp.tile([128, DT, S], bf16, name="kt_t")
            va_t = vap.tile([128, KT, H, HD + 1], bf16, name="va_t")
            store["qt"], store["kt"], store["va"] = qt_t, kt_t, va_t

        def qk_chunks(xt, ctt, store):
            """Q^T/K^T chunk closures, ordered (Q0,K0),(Q1,K1),... so pair
            hp only needs the first 2(hp+1) chunks."""
            chunks = []
            for m in range(DT):
                for wname, src, dstT in (("q", xt, store["qt"]), ("k", ctt, store["kt"])):
                    def f(wname=wname, src=src, dstT=dstT, m=m):
                        ps = psum_p.tile([128, 1024], f32, tag="big", name="ps_big")
                        for k in range(DT):
                            nc.tensor.matmul(
                                ps[:, 0:S],
                                lhsT=w_sb[wname][:, k, m * 128:(m + 1) * 128],
                                rhs=src[:, k, :],
                                start=(k == 0),
                                stop=(k == DT - 1),
                            )
                        nc.vector.tensor_scalar_add(
                            out=dstT[:, m, :],
                            in0=ps[:, 0:S],
                            scalar1=bias_sb[wname][:, m:m + 1],
                        )
                    chunks.append(f)
            return chunks

        def v_chunks(ctt, store):
            chunks = []
            va_t = store["va"]
            for m in range(KT):
                def f(m=m):
                    ps = psum_p.tile([128, 1024], f32, tag="big", name="ps_big")
                    for lo, hi in ((0, 512), (512, 768)):
                        for k in range(DT):
                            nc.tensor.matmul(
                                ps[:, lo:hi],
                                lhsT=ctt[:, k, m * 128:(m + 1) * 128],
                                rhs=w_sb["v"][:, k, lo:hi],
                                start=(k == 0),
                                stop=(k == DT - 1),
                            )
                    ps_h = ps[:, 0:D].rearrange("p (h x) -> p h x", x=HD)
                    nc.vector.tensor_add(out=va_t[:, m, :, 0:HD], in0=ps_h, in1=bv_sb)
                    nc.vector.memset(va_t[:, m, :, HD:HD + 1], 1.0)
                chunks.append(f)
            return chunks

        # ---- one attention head-pair for one batch ----
        def attn_pair(store, hp, orow):
            qt_t, kt_t, va_t = store["qt"], store["kt"], store["va"]
            exps_tiles = []
            for kt in range(KT):
                st = psum_p.tile([128, 2, S], f32, tag="big", name="st")
                for pr in (0, 1):
                    nc.tensor.matmul(
                        st[:, pr, :],
                        lhsT=kt_t[pr * 64:(pr + 1) * 64, hp, kt * 128:(kt + 1) * 128],
                        rhs=qt_t[pr * 64:(pr + 1) * 64, hp, :],
                        start=True,
                        stop=True,
                        tile_position=(pr * 64, 0),
                    )
                ex = exps_p.tile([128, 2, S], bf16, name="ex")
                nc.scalar.activation(out=ex, in_=st, func=AF.Exp, scale=0.125)
                exps_tiles.append(ex)
            for pr in (0, 1):
                h = 2 * hp + pr
                pv = pv_p.tile([128, QT, HD + 1], f32, tag="pv", name="pv")
                for q in range(QT):
                    for kt in range(KT):
                        nc.tensor.matmul(
                            pv[:, q, :],
                            lhsT=exps_tiles[kt][:, pr, q * 128:(q + 1) * 128],
                            rhs=va_t[:, kt, h, :],
                            start=(kt == 0),
                            stop=(kt == KT - 1),
                        )
                rc = small_p.tile([128, QT], f32, name="rc")
                nc.vector.reciprocal(
                    rc, pv[:, :, HD:HD + 1].rearrange("p a b -> p (a b)")
                )
                rc_b = bass.AP(
                    tensor=rc.tensor,
                    offset=rc.offset,
                    ap=[list(rc.ap[0]), [1, QT], [0, HD]],
                )
                nc.vector.tensor_mul(
                    out=orow[:, :, h * HD:(h + 1) * HD],
                    in0=pv[:, :, 0:HD],
                    in1=rc_b,
                )

        # ---- schedule: start attention as early as possible (ACT needs a
        #      long window); feed Q/K projection chunks just-in-time and
        #      spread batch-1 staging across batch-0's attention pairs ----
        stores = [{}, {}]
        x_nat0, c_nat0, xt0, ct0 = stage_loads(0)
        x_nat1, c_nat1, xt1, ct1 = stage_loads(1)
        for f in stage_chunks(x_nat0, c_nat0, xt0, ct0):
            f()
        emit_bias_transposes()
        proj_alloc(stores[0])
        proj_alloc(stores[1])
        for f in v_chunks(ct0, stores[0]):
            f()
        qk0 = qk_chunks(xt0, ct0, stores[0])
        qk0.pop(0)()
        qk0.pop(0)()

        fillers = stage_chunks(x_nat1, c_nat1, xt1, ct1) + v_chunks(
            ct1, stores[1]
        )

        orow0 = orow_p.tile([128, QT, D], f32, name="orow")
        for hp in range(HP):
            attn_pair(stores[0], hp, orow0)
            if qk0:
                qk0.pop(0)()
                qk0.pop(0)()
            for _ in range(3):
                if fillers:
                    fillers.pop(0)()
        while fillers:
            fillers.pop(0)()
        nc.sync.dma_start(
            out=out[0].rearrange("(q p) d -> p q d", p=128), in_=orow0
        )

        qk1 = qk_chunks(xt1, ct1, stores[1])
        qk1.pop(0)()
        qk1.pop(0)()
        orow1 = orow_p.tile([128, QT, D], f32, name="orow")
        for hp in range(HP):
            attn_pair(stores[1], hp, orow1)
            if qk1:
                qk1.pop(0)()
                qk1.pop(0)()
        nc.sync.dma_start(
            out=out[1].rearrange("(q p) d -> p q d", p=128), in_=orow1
        )


def build_program():
    if "nc" in _CACHE:
        return _CACHE["nc"]
    nc = bacc.Bacc("TRN2", target_bir_lowering=False, debug=False)
    hs = nc.dram_tensor("hs", [BL, S, D], f32, kind="ExternalInput").ap()
    ct = nc.dram_tensor("ct", [BL, S, D], f32, kind="ExternalInput").ap()
    w_aps = {
        n: nc.dram_tensor(f"w{n}", [D, D], f32, kind="ExternalInput").ap()
        for n in ("q", "k", "v")
    }
    b_aps = {
        n: nc.dram_tensor(f"b{n}", [D], f32, kind="ExternalInput").ap()
        for n in ("q", "k", "v")
    }
    out = nc.dram_tensor("out", [BL, S, D], f32, kind="ExternalOutput").ap()
    with tile.TileContext(nc) as tc:
        _emit(tc, hs, ct, w_aps, b_aps, out)
    nc.compile()
    _CACHE["nc"] = nc
    return nc


def make_in_maps(hidden_states, context, Wq, bq, Wk, bk, Wv, bv):
    hidden_states = np.ascontiguousarray(np.asarray(hidden_states, np.float32))
    context = np.ascontiguousarray(np.asarray(context, np.float32))
    common = {
        "wq": np.ascontiguousarray(np.asarray(Wq, np.float32)),
        "wk": np.ascontiguousarray(np.asarray(Wk, np.float32)),
        "wv": np.ascontiguousarray(np.asarray(Wv, np.float32)),
        "bq": np.ascontiguousarray(np.asarray(bq, np.float32)),
        "bk": np.ascontiguousarray(np.asarray(bk, np.float32)),
        "bv": np.ascontiguousarray(np.asarray(bv, np.float32)),
    }
    in_maps = []
    for c in range(NCORES):
        m = dict(common)
        m["hs"] = np.ascontiguousarray(hidden_states[c * BL:(c + 1) * BL])
        m["ct"] = np.ascontiguousarray(context[c * BL:(c + 1) * BL])
        in_maps.append(m)
    return in_maps


def run(in_maps, **kwargs):
    nc = build_program()
    return run_bass_kernel_spmd(nc, in_maps, core_ids=list(range(NCORES)), **kwargs)


def kernel(hidden_states, context, Wq, bq, Wk, bk, Wv, bv):
    in_maps = make_in_maps(hidden_states, context, Wq, bq, Wk, bk, Wv, bv)
    res = run(in_maps)
    outs = [np.asarray(res.results[i]["out"], np.float32) for i in range(NCORES)]
    return np.concatenate(outs, axis=0)
